# revision 1
# baseline (speedup 1.0000x reference)
"""Single-head causal attention (B=4, S=4096, d_e=512, d_k=d_v=64) on 8 TRN2 cores.

Strategy (SPMD, one program on all 8 cores; per-core behavior driven purely by data):
  - core c handles batch b=c//2; the two cores of a batch split the 8 q-tiles
    (512 queries each) in a load-balanced way: parity 0 -> q-tiles {0,2,5,7},
    parity 1 -> {1,3,4,6}  (equal causal work: 18 kv-tile interactions each).
  - x[b]^T is shipped per-core ([d_e, S], so projections need no on-device
    transpose). Projections compute k^T, v (with an appended ones column for the
    softmax denominator) for all 8 s-tiles, and q^T for the core's 4 q-tiles
    (read from x^T at data-driven dynamic offsets).
  - Attention runs in the "scores^T" layout: st[s,q] = k @ q^T so softmax sums
    ride the matmul (ones column of v_aug) and no transposes appear in the loop.
    Group g (q-tile slot g) statically processes kv s-tiles 0..2g+1; causal /
    padding masking is additive (-1e5) on the score PSUM, with mask tiles chosen
    from an SBUF palette at data-driven dynamic offsets (zero / keep / triangle).
  - exp on ACT writes float32r directly; all big matmuls run in float32r
    (single-pass PE mode, ~1e-4 rel err).
  - Finalize: PE-transpose out^T [65,512] -> [512,65], divide by the
    denominator column, DMA out. Host scatters per-core q-tiles into [B,S,64].
"""
import numpy as np
from contextlib import ExitStack

import concourse.bass as bass
import concourse.tile as tile
from concourse import bacc, mybir
from concourse.bass_utils import run_bass_kernel_spmd

f32 = mybir.dt.float32
f32r = mybir.dt.float32r
i32 = mybir.dt.int32
AF = mybir.ActivationFunctionType
ET = mybir.EngineType

B, S, DE, DK, DV = 4, 4096, 512, 64, 64
QT = 512                 # queries per group
NT = S // QT             # 8 s/q tiles per batch
NG = 4                   # groups (q-tiles) per core
NC_CHUNKS = DE // 128    # 4 contraction chunks
TQ = [[0, 2, 5, 7], [1, 3, 4, 6]]   # parity -> group -> q_tile index
MASK_NEG = -100000.0

# palette column offsets (elements): [drop(512) | keep(512) | tri master(896)]
PAL_DROP = 0
PAL_KEEP = 512
PAL_TRI0 = 1024 + 384    # tri for block blk is PAL_TRI0 - 128*blk


def build(kiter: int = 1):
    nc = bacc.Bacc("TRN2", target_bir_lowering=False, debug=False)

    xt_d = nc.dram_tensor("xt", [DE, S], f32r, kind="ExternalInput").ap()
    wkv_d = nc.dram_tensor("wkv", [DE, 128], f32r, kind="ExternalInput").ap()
    wq_d = nc.dram_tensor("wq", [DE, DK], f32r, kind="ExternalInput").ap()
    qoff_d = nc.dram_tensor("qoff", [1, 16], i32, kind="ExternalInput").ap()
    moff_d = nc.dram_tensor("moff", [1, 32], i32, kind="ExternalInput").ap()
    tri_d = nc.dram_tensor("tri", [128, 896], f32, kind="ExternalInput").ap()
    ident_d = nc.dram_tensor("ident", [128, 128], f32, kind="ExternalInput").ap()
    out_d = nc.dram_tensor("out", [NG, QT, DV], f32, kind="ExternalOutput").ap()

    with tile.TileContext(nc) as tc, ExitStack() as ctx:

        def body():
            per = ctx.enter_context(tc.tile_pool(name="persist", bufs=1))

            xts = per.tile([128, NC_CHUNKS * S], f32r)          # x^T stripes
            wkv = per.tile([128, NC_CHUNKS * 128], f32r)
            wq = per.tile([128, NC_CHUNKS * DK], f32r)
            ident = per.tile([128, 128], f32)
            pal = per.tile([128, 1920], f32)
            kT = per.tile([64, S], f32r)
            vaug = per.tile([128, (S // 128) * 65], f32r)       # 32 x [128,65]
            qTg = per.tile([64, NG * QT], f32r)
            qofft = per.tile([1, 16], i32)
            mofft = per.tile([1, 32], i32)

            for c in range(NC_CHUNKS):
                nc.sync.dma_start(xts[:, bass.ts(c, S)], xt_d[bass.ts(c, 128), :])
                nc.sync.dma_start(wkv[:, bass.ts(c, 128)], wkv_d[bass.ts(c, 128), :])
                nc.sync.dma_start(wq[:, bass.ts(c, DK)], wq_d[bass.ts(c, 128), :])
            nc.sync.dma_start(ident[:], ident_d[:])
            nc.sync.dma_start(qofft[:], qoff_d[:])
            nc.sync.dma_start(mofft[:], moff_d[:])
            nc.vector.memset(pal[:, 0:512], MASK_NEG)
            nc.vector.memset(pal[:, 512:1024], 0.0)
            nc.sync.dma_start(pal[:, 1024:1920], tri_d[:])

            qv = [nc.values_load(qofft[0:1, i:i + 1].to_broadcast((1, 1)),
                                 engines=[ET.PE], min_val=0, max_val=NC_CHUNKS * S - QT,
                                 skip_runtime_bounds_check=True)
                  for i in range(16)]
            mv = [nc.values_load(mofft[0:1, i:i + 1].to_broadcast((1, 1)),
                                 engines=[ET.DVE], min_val=0, max_val=1920 - 512,
                                 skip_runtime_bounds_check=True)
                  for i in range(32)]

            # ---- Phase 1: k^T / v projections for all 8 s-tiles -------------
            with tc.tile_pool(name="pkv", bufs=2, space="PSUM") as pkv_pool, \
                 tc.tile_pool(name="pvt", bufs=2, space="PSUM") as pvt_pool, \
                 tc.tile_pool(name="pq", bufs=2, space="PSUM") as pq_pool, \
                 tc.tile_pool(name="vts", bufs=2) as vts_pool:
                for t in range(NT):
                    pkv = pkv_pool.tile([128, QT], f32)
                    for c in range(NC_CHUNKS):
                        nc.tensor.matmul(pkv[:], wkv[:, bass.ts(c, 128)],
                                         xts[:, c * S + t * QT: c * S + (t + 1) * QT],
                                         start=(c == 0), stop=(c == NC_CHUNKS - 1))
                    nc.vector.tensor_copy(kT[:, bass.ts(t, QT)], pkv[0:64, :])
                    vts = vts_pool.tile([65, QT], f32)
                    nc.vector.tensor_copy(vts[0:64, :], pkv[64:128, :])
                    nc.vector.memset(vts[64:65, :], 1.0)
                    for blk in range(4):
                        pvt = pvt_pool.tile([128, 65], f32)
                        nc.tensor.transpose(pvt[:], vts[:, bass.ts(blk, 128)],
                                            ident[0:65, 0:65])
                        sb = 4 * t + blk
                        nc.vector.tensor_copy(vaug[:, sb * 65:(sb + 1) * 65], pvt[:])

                # ---- Phase 2: q^T for the core's 4 q-tiles (dynamic offsets) --
                for g in range(NG):
                    pq = pq_pool.tile([64, QT], f32)
                    for c in range(NC_CHUNKS):
                        nc.tensor.matmul(pq[:], wq[:, bass.ts(c, DK)],
                                         xts[:, bass.ds(qv[g * 4 + c], QT)],
                                         start=(c == 0), stop=(c == NC_CHUNKS - 1))
                    nc.vector.tensor_copy(qTg[:, bass.ts(g, QT)], pq[:])

            # ---- Phase 3: attention ----------------------------------------
            with tc.tile_pool(name="ps", bufs=2, space="PSUM") as ps_pool, \
                 tc.tile_pool(name="po", bufs=2, space="PSUM") as po_pool, \
                 tc.tile_pool(name="pt", bufs=2, space="PSUM") as pt_pool, \
                 tc.tile_pool(name="exp", bufs=3) as exp_pool, \
                 tc.tile_pool(name="fin", bufs=2) as fin_pool:
                for g in range(NG):
                    npairs = 4 * g + 4
                    nblocks = 8 * g + 8
                    po = po_pool.tile([65, QT], f32)
                    pending = None  # (psumS, expM-slot pair index)

                    def flush(pend):
                        ps, pi = pend
                        if pi >= 4 * g:  # masked pair: additive mask on scores
                            for half in range(2):
                                rel = 2 * pi + half - 8 * g
                                nc.vector.tensor_add(
                                    ps[:, bass.ts(half, QT)], ps[:, bass.ts(half, QT)],
                                    pal[:, bass.ds(mv[g * 8 + rel], QT)])
                        em = exp_pool.tile([128, 2 * QT], f32r)
                        nc.scalar.activation(em[:], ps[:], AF.Exp)
                        for half in range(2):
                            sb = 2 * pi + half
                            nc.tensor.matmul(po[:], vaug[:, (sb) * 65:(sb + 1) * 65],
                                             em[:, bass.ts(half, QT)],
                                             start=(sb == 0), stop=(sb == nblocks - 1))

                    for pi in range(npairs):
                        ps = ps_pool.tile([128, 2 * QT], f32)
                        for half in range(2):
                            sb = 2 * pi + half
                            nc.tensor.matmul(ps[:, bass.ts(half, QT)],
                                             kT[:, bass.ts(sb, 128)],
                                             qTg[:, bass.ts(g, QT)],
                                             start=True, stop=True)
                        if pending is not None:
                            flush(pending)
                        pending = (ps, pi)
                    flush(pending)

                    # ---- finalize group g ----------------------------------
                    oT = fin_pool.tile([65, QT], f32)
                    nc.vector.tensor_copy(oT[:], po[:])
                    for blk in range(4):
                        pt = pt_pool.tile([128, 65], f32)
                        nc.tensor.transpose(pt[:], oT[:, bass.ts(blk, 128)],
                                            ident[0:65, 0:65])
                        onat = fin_pool.tile([128, 65], f32)
                        nc.vector.tensor_copy(onat[:], pt[:])
                        rec = fin_pool.tile([128, 1], f32)
                        nc.vector.reciprocal(rec[:], onat[:, 64:65])
                        ofin = fin_pool.tile([128, DV], f32)
                        nc.vector.tensor_scalar_mul(ofin[:], onat[:, 0:64], rec[:])
                        nc.sync.dma_start(out_d[g, bass.ts(blk, 128), :], ofin[:])

        if kiter == 1:
            body()
        else:
            with tc.For_i(0, kiter, 1):
                body()

    nc.compile()
    return nc


def make_inputs(x, Wq, Wk, Wv):
    """Per-core input maps. x:[B,S,DE] f32; W*: [DE,64] f32."""
    wkv = np.ascontiguousarray(np.concatenate([Wk, Wv], axis=1), dtype=np.float32)
    wqs = np.ascontiguousarray(Wq / np.float32(np.sqrt(DK)), dtype=np.float32)
    ident = np.eye(128, dtype=np.float32)
    tri = np.where(np.arange(896)[None, :] >= np.arange(128)[:, None] + 384,
                   np.float32(0.0), np.float32(MASK_NEG)).astype(np.float32)
    in_maps = []
    for core in range(8):
        b, p = core // 2, core % 2
        xt = np.ascontiguousarray(x[b].T, dtype=np.float32)
        qoff = np.zeros((1, 16), dtype=np.int32)
        for g in range(NG):
            for c in range(NC_CHUNKS):
                qoff[0, g * 4 + c] = c * S + TQ[p][g] * QT
        moff = np.zeros((1, 32), dtype=np.int32)
        for g in range(NG):
            t = TQ[p][g]
            for rel in range(8):
                j = 2 * g + rel // 4
                blk = rel % 4
                if j < t:
                    moff[0, g * 8 + rel] = PAL_KEEP
                elif j == t:
                    moff[0, g * 8 + rel] = PAL_TRI0 - 128 * blk
                else:
                    moff[0, g * 8 + rel] = PAL_DROP
        in_maps.append(dict(xt=xt, wkv=wkv, wq=wqs, qoff=qoff, moff=moff,
                            tri=tri, ident=ident))
    return in_maps


def assemble(results):
    out = np.empty((B, S, DV), dtype=np.float32)
    for core in range(8):
        b, p = core // 2, core % 2
        o = results[core]["out"]
        for g in range(NG):
            t = TQ[p][g]
            out[b, t * QT:(t + 1) * QT, :] = o[g]
    return out


_cache = {}


def _get_nc(kiter=1):
    if kiter not in _cache:
        _cache[kiter] = build(kiter)
    return _cache[kiter]


def run(x, Wq, Wk, Wv, kiter=1):
    nc = _get_nc(kiter)
    in_maps = make_inputs(x, Wq, Wk, Wv)
    res = run_bass_kernel_spmd(nc, in_maps, list(range(8)))
    return assemble(res.results)


def kernel(x, Wq, Wk, Wv):
    x = np.asarray(x, dtype=np.float32)
    return run(x, np.asarray(Wq, np.float32), np.asarray(Wk, np.float32),
               np.asarray(Wv, np.float32))


# revision 12
# speedup vs baseline: 1.1894x; 1.1894x over previous
"""Single-head causal attention (B=4, S=4096, d_e=512, d_k=d_v=64) on 8 TRN2 cores.

SPMD: one program on all 8 cores; per-core behavior driven purely by input data.
  - core c handles batch b=c//2; the two cores of a batch split the 8 q-tiles
    (512 queries each) load-balanced: parity 0 -> q-tiles {0,2,5,7}, parity 1 ->
    {1,3,4,6} (equal causal work: 18 kv-tile interactions each, padded to 20).
  - Inputs are host-tiled so each s-tile / q-tile arrives in ONE DMA (DMA issue
    costs ~1.2us of sequencer time each); issue is spread across SP/DVE/ACT
    queues so transfers pipeline early.
  - Attention in "scores^T" layout: st[s,q] = k @ (q/sqrt(dk))^T. The softmax
    denominator rides the AV matmul via an appended ones column on v. Causal /
    padding masks are multiplicative {0,1} tiles picked from an SBUF palette at
    data-driven dynamic offsets, applied post-exp on GPSIMD (off critical
    engines). Masked pairs run FIRST within each group so the group tail is
    mask-free.
  - All large matmuls in float32r (single-pass PE, ~1e-4 rel err); score pairs
    are row-stacked on the PE array via tile_position (K=64 each) so two score
    matmuls run concurrently.
  - Finalize: PE-transpose out^T [65,512] -> [512,65], multiply by reciprocal
    of the denominator column, one DMA per group. Host scatters into [B,S,64].
"""
import numpy as np
from contextlib import ExitStack

import concourse.bass as bass
import concourse.tile as tile
from concourse import bacc, mybir
from concourse.tile import add_dep_helper
from concourse.bass_utils import run_bass_kernel_spmd

f32 = mybir.dt.float32
f32r = mybir.dt.float32r
i32 = mybir.dt.int32
AF = mybir.ActivationFunctionType
ET = mybir.EngineType

B, S, DE, DK, DV = 4, 4096, 512, 64, 64
QT = 512                 # queries per group
NT = S // QT             # 8 s/q tiles per batch
NG = 4                   # groups (q-tiles) per core
NCH = DE // 128          # 4 contraction chunks
TW = NCH * QT            # tile width in sbuf cols (2048)
TQ = [[0, 2, 5, 7], [1, 3, 4, 6]]   # parity -> group -> q_tile index

# palette column offsets (elements): [drop(512) | keep(512) | tri master(896)]
PAL_DROP = 0
PAL_KEEP = 512
PAL_TRI0 = 1024 + 384    # tri for block blk is PAL_TRI0 - 128*blk


def build(kiter: int = 1):
    nc = bacc.Bacc("TRN2", target_bir_lowering=False, debug=False)

    xt_d = nc.dram_tensor("xt", [NT, 128, TW], f32r, kind="ExternalInput").ap()
    xq_d = nc.dram_tensor("xq", [NG, 128, TW], f32r, kind="ExternalInput").ap()
    wkv_d = nc.dram_tensor("wkv", [128, NCH * 128], f32r, kind="ExternalInput").ap()
    wq_d = nc.dram_tensor("wq", [128, NCH * DK], f32r, kind="ExternalInput").ap()
    moff_d = nc.dram_tensor("moff", [1, 32], i32, kind="ExternalInput").ap()
    tri_d = nc.dram_tensor("tri", [128, 896], f32r, kind="ExternalInput").ap()
    ident_d = nc.dram_tensor("ident", [128, 128], f32, kind="ExternalInput").ap()
    out_d = nc.dram_tensor("out", [NG, 128, 4 * DV], f32, kind="ExternalOutput").ap()

    with tile.TileContext(nc) as tc, ExitStack() as ctx:

        def body():
            per = ctx.enter_context(tc.tile_pool(name="persist", bufs=1))
            # psum pools: 1 + 1 + 1 + 4 + 1 = 8 banks, all coexist (no
            # released-zone overlap deps between phases)
            pkv_pool = ctx.enter_context(tc.tile_pool(name="pkv", bufs=2, space="PSUM"))
            pq_pool = ctx.enter_context(tc.tile_pool(name="pq", bufs=2, space="PSUM"))
            ps_pool = ctx.enter_context(tc.tile_pool(name="ps", bufs=2, space="PSUM"))

            vts_pool = ctx.enter_context(tc.tile_pool(name="vts", bufs=2))
            exp_pool = ctx.enter_context(tc.tile_pool(name="exp", bufs=8))
            fin_pool = ctx.enter_context(tc.tile_pool(name="fin", bufs=2))

            xts = per.tile([128, NT * TW], f32r)           # x^T, tile-major
            xqs = per.tile([128, NG * TW], f32r)           # x^T own q-tiles
            wkv = per.tile([128, NCH * 128], f32r)
            wq = per.tile([128, NCH * DK], f32r)
            ident = per.tile([128, 128], f32)
            pal = per.tile([128, 1920], f32r)
            kT = per.tile([128, S], f32r)   # rows 0:64 and 64:128 both hold k^T
            vaug = per.tile([128, (S // 128) * 65], f32r)  # 32 x [128,65]
            qTg = per.tile([128, NG * QT], f32r)  # duplicated rows like kT
            mofft = per.tile([1, 32], i32)

            # constants + weights on ACT queue (idle early)
            nc.scalar.dma_start(mofft[:], moff_d[:])
            nc.scalar.dma_start(wkv[:], wkv_d[:])
            nc.scalar.dma_start(wq[:], wq_d[:])
            nc.scalar.dma_start(ident[:], ident_d[:])
            nc.scalar.dma_start(pal[:, 1024:1920], tri_d[:])
            nc.gpsimd.memset(pal[:, 0:512].bitcast(f32), 0.0)
            nc.gpsimd.memset(pal[:, 512:1024].bitcast(f32), 1.0)

            # xq on DVE queue, xt on SP queue; issue order interleaves so the
            # early tiles of both are resident quickly
            for g in range(NG):
                nc.scalar.dma_start(xqs[:, bass.ts(g, TW)], xq_d[g])
                nc.sync.dma_start(xts[:, bass.ts(2 * g, TW)], xt_d[2 * g])
                nc.sync.dma_start(xts[:, bass.ts(2 * g + 1, TW)], xt_d[2 * g + 1])

            mv = [nc.values_load(mofft[0:1, i:i + 1].to_broadcast((1, 1)),
                                 engines=[ET.Pool, ET.DVE], min_val=0, max_val=1920 - 512,
                                 skip_runtime_bounds_check=True)
                  for i in range(32)]

            # ---- projections ------------------------------------------------
            # q^T for the core's 4 q-tiles
            for g in range(NG):
                pq_t = pq_pool.tile([65, QT], f32, tag="pqo")
                pq = pq_t[0:64, :]
                for c in range(NCH):
                    nc.tensor.matmul(pq[:], wq[:, bass.ts(c, DK)],
                                     xqs[:, g * TW + c * QT: g * TW + (c + 1) * QT],
                                     start=(c == 0), stop=(c == NCH - 1))
                nc.vector.tensor_copy(qTg[0:64, bass.ts(g, QT)], pq[:])
                nc.vector.tensor_copy(qTg[64:128, bass.ts(g, QT)], pq[:])

            # k^T and v_aug for one s-tile
            def kv_proj(t):
                pkv = pkv_pool.tile([128, QT], f32, tag="pkvt")
                for c in range(NCH):
                    nc.tensor.matmul(pkv[:], wkv[:, bass.ts(c, 128)],
                                     xts[:, t * TW + c * QT: t * TW + (c + 1) * QT],
                                     start=(c == 0), stop=(c == NCH - 1))
                nc.vector.tensor_copy(kT[0:64, bass.ts(t, QT)], pkv[0:64, :])
                nc.vector.tensor_copy(kT[64:128, bass.ts(t, QT)], pkv[0:64, :])
                vts = vts_pool.tile([65, QT], f32, tag="vts")
                nc.vector.tensor_copy(vts[0:64, :], pkv[64:128, :])
                nc.vector.memset(vts[64:65, :], 1.0)
                pvt = pkv_pool.tile([128, 4 * 65], f32, tag="pkvt")
                for blk in range(4):
                    nc.tensor.transpose(pvt[:, bass.ts(blk, 65)],
                                        vts[:, bass.ts(blk, 128)],
                                        ident[0:65, 0:65])
                nc.vector.tensor_copy(vaug[:, t * 4 * 65:(t + 1) * 4 * 65], pvt[:])

            # ---- attention, kv-projections interleaved just-in-time ---------
            for g in range(NG):
                kv_proj(2 * g)
                kv_proj(2 * g + 1)
                npairs = 4 * g + 4
                po = pq_pool.tile([65, QT], f32, tag="pqo")
                # masked pairs (the last 4 in index space) are computed first
                # (scores+exp+mask) but their AV matmuls are deferred to the
                # group tail so mask-multiply latency stays off the PE chain.
                order = list(range(4 * g, 4 * g + 4)) + list(range(4 * g))
                av_emitted = [0]
                n_av = 2 * npairs
                deferred = []

                def emit_av(pi, em, g=g, po=po):
                    for half in range(2):
                        sb = 2 * pi + half
                        nc.tensor.matmul(po[:], vaug[:, sb * 65:(sb + 1) * 65],
                                         em[:, bass.ts(half, QT)],
                                         start=(av_emitted[0] == 0),
                                         stop=(av_emitted[0] == n_av - 2))
                        av_emitted[0] += 2

                def flush(pend, g=g):
                    ps, pi = pend
                    em = exp_pool.tile([128, 2 * QT], f32r)
                    nc.scalar.activation(em[:], ps[:], AF.Exp)
                    if pi >= 4 * g:   # masked pair: mults split DVE/POOL
                        for half in range(2):
                            rel = 2 * pi + half - 8 * g
                            eng = nc.vector if half == 0 else nc.gpsimd
                            eng.tensor_mul(
                                em[:, bass.ts(half, QT)],
                                em[:, bass.ts(half, QT)],
                                pal[:, bass.ds(mv[g * 8 + rel], QT)])
                        deferred.append((pi, em))
                    else:
                        emit_av(pi, em)

                pending = None
                for pi in order:
                    ps = ps_pool.tile([128, 2 * QT], f32)
                    for half in range(2):
                        sb = 2 * pi + half
                        rows = slice(64 * half, 64 * half + 64)
                        nc.tensor.matmul(ps[:, bass.ts(half, QT)],
                                         kT[rows, bass.ts(sb, 128)],
                                         qTg[rows, bass.ts(g, QT)],
                                         start=True, stop=True,
                                         tile_position=(64 * half, 0))
                    if pending is not None:
                        flush(pending)
                    pending = (ps, pi)
                flush(pending)
                for pi, em in deferred:
                    emit_av(pi, em)

                # finalize group g
                oT = fin_pool.tile([65, QT], f32)
                nc.vector.tensor_copy(oT[:], po[:])
                pt = pkv_pool.tile([128, 4 * 65], f32, tag="pkvt")
                for blk in range(4):
                    nc.tensor.transpose(pt[:, bass.ts(blk, 65)],
                                        oT[:, bass.ts(blk, 128)],
                                        ident[0:65, 0:65])
                onat = fin_pool.tile([128, 4 * 65], f32)
                nc.vector.tensor_copy(onat[:], pt[:])
                ofin = fin_pool.tile([128, 4 * DV], f32)
                for blk in range(4):
                    rec = fin_pool.tile([128, 1], f32)
                    nc.vector.reciprocal(rec[:], onat[:, blk * 65 + 64: blk * 65 + 65])
                    nc.vector.tensor_scalar_mul(
                        ofin[:, bass.ts(blk, DV)], onat[:, blk * 65: blk * 65 + 64],
                        rec[:])
                nc.sync.dma_start(out_d[g], ofin[:])

        if kiter == 1:
            body()
        else:
            with tc.For_i(0, kiter, 1):
                body()

    nc.compile()
    return nc


def _tile_cols(a):
    """[512, n*512] (d_e, cols) -> [n, 128, 4*512] tile-major host layout."""
    de, w = a.shape
    n = w // QT
    # out[t, p, c*QT + s] = a[c*128 + p, t*QT + s]
    return np.ascontiguousarray(
        a.reshape(NCH, 128, n, QT).transpose(2, 1, 0, 3).reshape(n, 128, NCH * QT))


def make_inputs(x, Wq, Wk, Wv):
    """Per-core input maps. x:[B,S,DE] f32; W*: [DE,64] f32."""
    wkv = np.concatenate([Wk, Wv], axis=1).astype(np.float32)          # [512,128]
    wqs = (Wq / np.float32(np.sqrt(DK))).astype(np.float32)            # [512,64]
    # weights chunk-major: [128, c*width + j] = W[c*128 + p, j]
    wkv_h = np.ascontiguousarray(
        wkv.reshape(NCH, 128, 128).transpose(1, 0, 2).reshape(128, NCH * 128))
    wq_h = np.ascontiguousarray(
        wqs.reshape(NCH, 128, DK).transpose(1, 0, 2).reshape(128, NCH * DK))
    ident = np.eye(128, dtype=np.float32)
    tri = (np.arange(896)[None, :] >= np.arange(128)[:, None] + 384).astype(np.float32)
    in_maps = []
    for core in range(8):
        b, p = core // 2, core % 2
        xt = np.ascontiguousarray(x[b].T, dtype=np.float32)            # [512, 4096]
        cols = np.concatenate([np.arange(t * QT, (t + 1) * QT) for t in TQ[p]])
        moff = np.zeros((1, 32), dtype=np.int32)
        for g in range(NG):
            t = TQ[p][g]
            for rel in range(8):
                j = 2 * g + rel // 4
                blk = rel % 4
                if j < t:
                    moff[0, g * 8 + rel] = PAL_KEEP
                elif j == t:
                    moff[0, g * 8 + rel] = PAL_TRI0 - 128 * blk
                else:
                    moff[0, g * 8 + rel] = PAL_DROP
        in_maps.append(dict(xt=_tile_cols(xt), xq=_tile_cols(xt[:, cols]),
                            wkv=wkv_h, wq=wq_h, moff=moff, tri=tri, ident=ident))
    return in_maps


def assemble(results):
    out = np.empty((B, S, DV), dtype=np.float32)
    for core in range(8):
        b, p = core // 2, core % 2
        o = results[core]["out"]                      # [NG, 128, 4*64]
        for g in range(NG):
            t = TQ[p][g]
            # query q = blk*128 + p_row lives at o[g][p_row, blk*64:(blk+1)*64]
            blk_view = o[g].reshape(128, 4, DV).transpose(1, 0, 2)   # [blk,p,dv]
            out[b, t * QT:(t + 1) * QT, :] = blk_view.reshape(QT, DV)
    return out


_cache = {}


def _get_nc(kiter=1):
    if kiter not in _cache:
        _cache[kiter] = build(kiter)
    return _cache[kiter]


def run(x, Wq, Wk, Wv, kiter=1):
    nc = _get_nc(kiter)
    in_maps = make_inputs(x, Wq, Wk, Wv)
    res = run_bass_kernel_spmd(nc, in_maps, list(range(8)))
    return assemble(res.results)


def kernel(x, Wq, Wk, Wv):
    x = np.asarray(x, dtype=np.float32)
    return run(x, np.asarray(Wq, np.float32), np.asarray(Wk, np.float32),
               np.asarray(Wv, np.float32))


# revision 18
# speedup vs baseline: 1.6174x; 1.3599x over previous
"""Single-head causal attention (B=4, S=4096, d_e=512, d_k=d_v=64) on 8 TRN2 cores.

SPMD: one program on all 8 cores; per-core behavior driven purely by input data.
  - core c handles batch b=c//2; the two cores of a batch split the 8 q-tiles
    (512 queries each) load-balanced: parity 0 -> q-tiles {0,2,5,7}, parity 1 ->
    {1,3,4,6} (equal causal work: 18 kv-tile interactions each, padded to 20).
  - Inputs are host-tiled so each s-tile / q-tile arrives in ONE DMA (DMA issue
    costs ~1.2us of sequencer time each); issue is spread across SP/DVE/ACT
    queues so transfers pipeline early.
  - Attention in "scores^T" layout: st[s,q] = k @ (q/sqrt(dk))^T. The softmax
    denominator rides the AV matmul via an appended ones column on v. Causal /
    padding masks are multiplicative {0,1} tiles picked from an SBUF palette at
    data-driven dynamic offsets, applied post-exp on GPSIMD (off critical
    engines). Masked pairs run FIRST within each group so the group tail is
    mask-free.
  - All large matmuls in float32r (single-pass PE, ~1e-4 rel err); score pairs
    are row-stacked on the PE array via tile_position (K=64 each) so two score
    matmuls run concurrently.
  - Finalize: PE-transpose out^T [65,512] -> [512,65], multiply by reciprocal
    of the denominator column, one DMA per group. Host scatters into [B,S,64].
"""
import numpy as np
from contextlib import ExitStack

import concourse.bass as bass
import concourse.tile as tile
from concourse import bacc, mybir
from concourse.tile import add_dep_helper
from concourse.bass_utils import run_bass_kernel_spmd

f32 = mybir.dt.float32
f32r = mybir.dt.float32r
i32 = mybir.dt.int32
AF = mybir.ActivationFunctionType
ET = mybir.EngineType

B, S, DE, DK, DV = 4, 4096, 512, 64, 64
QT = 512                 # queries per group
NT = S // QT             # 8 s/q tiles per batch
NG = 4                   # groups (q-tiles) per core
NCH = DE // 128          # 4 contraction chunks
TW = NCH * QT            # tile width in sbuf cols (2048)
TQ = [[0, 2, 5, 7], [1, 3, 4, 6]]   # parity -> group -> q_tile index

# palette column offsets (elements): [drop(512) | keep(512) | tri master(896)]
PAL_DROP = 0
PAL_KEEP = 512
PAL_TRI0 = 1024 + 384    # tri for block blk is PAL_TRI0 - 128*blk


def build(kiter: int = 1):
    nc = bacc.Bacc("TRN2", target_bir_lowering=False, debug=False)

    xt_d = nc.dram_tensor("xt", [NT, 128, TW], f32r, kind="ExternalInput").ap()
    xq_d = nc.dram_tensor("xq", [NG, 128, TW], f32r, kind="ExternalInput").ap()
    wkv_d = nc.dram_tensor("wkv", [128, NCH * 128], f32r, kind="ExternalInput").ap()
    wq_d = nc.dram_tensor("wq", [128, NCH * DK], f32r, kind="ExternalInput").ap()
    moff_d = nc.dram_tensor("moff", [1, 32], i32, kind="ExternalInput").ap()
    tri_d = nc.dram_tensor("tri", [128, 896], f32r, kind="ExternalInput").ap()
    ident_d = nc.dram_tensor("ident", [128, 128], f32, kind="ExternalInput").ap()
    out_d = nc.dram_tensor("out", [NG, 128, 4 * DV], f32, kind="ExternalOutput").ap()

    with tile.TileContext(nc) as tc, ExitStack() as ctx:

        def body():
            per = ctx.enter_context(tc.tile_pool(name="persist", bufs=1))
            # PSUM pools: pkvt 2x[128,512] + pqo 2x[65,512] + ps 2x[128,1024]
            # = 8 banks, all coexisting (no cross-phase overlap deps)
            pkv_pool = ctx.enter_context(tc.tile_pool(name="pkv", bufs=2, space="PSUM"))
            pq_pool = ctx.enter_context(tc.tile_pool(name="pq", bufs=2, space="PSUM"))
            ps_pool = ctx.enter_context(tc.tile_pool(name="ps", bufs=2, space="PSUM"))

            vts_pool = ctx.enter_context(tc.tile_pool(name="vts", bufs=2))
            exp_pool = ctx.enter_context(tc.tile_pool(name="exp", bufs=8))
            fin_pool = ctx.enter_context(tc.tile_pool(name="fin", bufs=2))

            xts = per.tile([128, NT * TW], f32r)           # x^T, tile-major
            xqs = per.tile([128, NG * TW], f32r)           # x^T own q-tiles
            wkv = per.tile([128, NCH * 128], f32r)
            wq = per.tile([128, NCH * DK], f32r)
            ident = per.tile([128, 128], f32)
            pal = per.tile([128, 1920], f32r)
            kT = per.tile([128, S], f32r)   # rows 0:64 and 64:128 both hold k^T
            vaug = per.tile([128, (S // 128) * 65], f32r)  # 32 x [128,65]
            qTg = per.tile([128, NG * QT], f32r)  # duplicated rows like kT
            oTall = per.tile([65, NG * QT], f32)
            mofft = per.tile([1, 32], i32)

            # weights/constants on ACT queue; xq0 + all xt on SP queue so the
            # first group's data lands earliest
            nc.sync.dma_start(xqs[:, bass.ts(0, TW)], xq_d[0])
            nc.scalar.dma_start(wkv[:], wkv_d[:])
            nc.scalar.dma_start(wq[:], wq_d[:])
            nc.scalar.dma_start(mofft[:], moff_d[:])
            nc.scalar.dma_start(ident[:], ident_d[:])
            nc.scalar.dma_start(pal[:, 1024:1920], tri_d[:])
            for g in range(1, NG):
                nc.scalar.dma_start(xqs[:, bass.ts(g, TW)], xq_d[g])
            for t in range(NT):
                nc.sync.dma_start(xts[:, bass.ts(t, TW)], xt_d[t])
            nc.gpsimd.memset(pal[:, 0:512].bitcast(f32), 0.0)
            nc.gpsimd.memset(pal[:, 512:1024].bitcast(f32), 1.0)

            mv = [nc.values_load(mofft[0:1, i:i + 1].to_broadcast((1, 1)),
                                 engines=[ET.Pool, ET.DVE], min_val=0, max_val=1920 - 512,
                                 skip_runtime_bounds_check=True)
                  for i in range(32)]

            # ---- projections ------------------------------------------------
            def q_proj(g):
                pq_t = pq_pool.tile([65, QT], f32, tag="pqo")
                pq = pq_t[0:64, :]
                for c in range(NCH):
                    nc.tensor.matmul(pq[:], wq[:, bass.ts(c, DK)],
                                     xqs[:, g * TW + c * QT: g * TW + (c + 1) * QT],
                                     start=(c == 0), stop=(c == NCH - 1))
                nc.vector.tensor_copy(qTg[0:64, bass.ts(g, QT)], pq[:])
                nc.vector.tensor_copy(qTg[64:128, bass.ts(g, QT)], pq[:])

            # k^T and v_aug for one s-tile
            def kv_proj(t):
                pkv = pkv_pool.tile([128, QT], f32, tag="pkvt")
                for c in range(NCH):
                    nc.tensor.matmul(pkv[:], wkv[:, bass.ts(c, 128)],
                                     xts[:, t * TW + c * QT: t * TW + (c + 1) * QT],
                                     start=(c == 0), stop=(c == NCH - 1))
                nc.vector.tensor_copy(kT[0:64, bass.ts(t, QT)], pkv[0:64, :])
                nc.vector.tensor_copy(kT[64:128, bass.ts(t, QT)], pkv[0:64, :])
                vts = vts_pool.tile([65, QT], f32, tag="vts")
                nc.vector.tensor_copy(vts[0:64, :], pkv[64:128, :])
                nc.vector.memset(vts[64:65, :], 1.0)
                pvt = pkv_pool.tile([128, 4 * 65], f32, tag="pkvt")
                for blk in range(4):
                    nc.tensor.transpose(pvt[:, bass.ts(blk, 65)],
                                        vts[:, bass.ts(blk, 128)],
                                        ident[0:65, 0:65])
                nc.vector.tensor_copy(vaug[:, t * 4 * 65:(t + 1) * 4 * 65], pvt[:])

            for g in range(NG):
                q_proj(g)

            # ---- attention, group-major, kv-projections just-in-time --------
            for g in range(NG):
                kv_proj(2 * g)
                kv_proj(2 * g + 1)
                npairs = 4 * g + 4
                po = pq_pool.tile([65, QT], f32, tag="pqo")
                # masked pairs (last 4 in index space) run first; their AV
                # matmuls are deferred to the group tail so the mask-multiply
                # latency stays off the PE chain.
                order = list(range(4 * g, 4 * g + 4)) + list(range(4 * g))
                av_emitted = [0]
                n_av = 2 * npairs
                deferred = []

                def emit_av(pi, em, g=g, po=po, av_emitted=av_emitted, n_av=n_av):
                    for half in range(2):
                        sb = 2 * pi + half
                        nc.tensor.matmul(po[:], vaug[:, sb * 65:(sb + 1) * 65],
                                         em[:, bass.ts(half, QT)],
                                         start=(av_emitted[0] == 0),
                                         stop=(av_emitted[0] == n_av - 2))
                        av_emitted[0] += 2

                def flush(pend, g=g, emit_av=emit_av, deferred=deferred):
                    ps, pi = pend
                    em = exp_pool.tile([128, 2 * QT], f32r)
                    nc.scalar.activation(em[:], ps[:], AF.Exp)
                    if pi >= 4 * g:   # masked pair: mults split DVE/POOL
                        for half in range(2):
                            rel = 2 * pi + half - 8 * g
                            eng = nc.vector if half == 0 else nc.gpsimd
                            eng.tensor_mul(
                                em[:, bass.ts(half, QT)],
                                em[:, bass.ts(half, QT)],
                                pal[:, bass.ds(mv[g * 8 + rel], QT)])
                        deferred.append((pi, em))
                    else:
                        emit_av(pi, em)

                pending = None
                for pi in order:
                    ps = ps_pool.tile([128, 2 * QT], f32)
                    for half in range(2):
                        sb = 2 * pi + half
                        rows = slice(64 * half, 64 * half + 64)
                        nc.tensor.matmul(ps[:, bass.ts(half, QT)],
                                         kT[rows, bass.ts(sb, 128)],
                                         qTg[rows, bass.ts(g, QT)],
                                         start=True, stop=True,
                                         tile_position=(64 * half, 0))
                    if pending is not None:
                        flush(pending)
                    pending = (ps, pi)
                flush(pending)
                for pi, em in deferred:
                    emit_av(pi, em)

                # release po immediately; heavy finalize deferred to the tail
                nc.vector.tensor_copy(oTall[:, bass.ts(g, QT)], po[:])

            # ---- finalize tail: transpose, normalize, store ----------------
            for g in range(NG):
                pt = pkv_pool.tile([128, 4 * 65], f32, tag="pkvt")
                for blk in range(4):
                    nc.tensor.transpose(pt[:, bass.ts(blk, 65)],
                                        oTall[:, g * QT + blk * 128:
                                              g * QT + (blk + 1) * 128],
                                        ident[0:65, 0:65])
                onat = fin_pool.tile([128, 4 * 65], f32)
                nc.vector.tensor_copy(onat[:], pt[:])
                ofin = fin_pool.tile([128, 4 * DV], f32)
                for blk in range(4):
                    rec = fin_pool.tile([128, 1], f32)
                    nc.vector.reciprocal(rec[:], onat[:, blk * 65 + 64: blk * 65 + 65])
                    nc.vector.tensor_scalar_mul(
                        ofin[:, bass.ts(blk, DV)], onat[:, blk * 65: blk * 65 + 64],
                        rec[:])
                nc.sync.dma_start(out_d[g], ofin[:])

        if kiter == 1:
            body()
        else:
            with tc.For_i(0, kiter, 1):
                body()

    nc.compile()
    return nc


def _tile_cols(a):
    """[512, n*512] (d_e, cols) -> [n, 128, 4*512] tile-major host layout."""
    de, w = a.shape
    n = w // QT
    # out[t, p, c*QT + s] = a[c*128 + p, t*QT + s]
    return np.ascontiguousarray(
        a.reshape(NCH, 128, n, QT).transpose(2, 1, 0, 3).reshape(n, 128, NCH * QT))


def make_inputs(x, Wq, Wk, Wv):
    """Per-core input maps. x:[B,S,DE] f32; W*: [DE,64] f32."""
    wkv = np.concatenate([Wk, Wv], axis=1).astype(np.float32)          # [512,128]
    wqs = (Wq / np.float32(np.sqrt(DK))).astype(np.float32)            # [512,64]
    # weights chunk-major: [128, c*width + j] = W[c*128 + p, j]
    wkv_h = np.ascontiguousarray(
        wkv.reshape(NCH, 128, 128).transpose(1, 0, 2).reshape(128, NCH * 128))
    wq_h = np.ascontiguousarray(
        wqs.reshape(NCH, 128, DK).transpose(1, 0, 2).reshape(128, NCH * DK))
    ident = np.eye(128, dtype=np.float32)
    tri = (np.arange(896)[None, :] >= np.arange(128)[:, None] + 384).astype(np.float32)
    in_maps = []
    for core in range(8):
        b, p = core // 2, core % 2
        xt = np.ascontiguousarray(x[b].T, dtype=np.float32)            # [512, 4096]
        cols = np.concatenate([np.arange(t * QT, (t + 1) * QT) for t in TQ[p]])
        moff = np.zeros((1, 32), dtype=np.int32)
        for g in range(NG):
            t = TQ[p][g]
            for rel in range(8):
                j = 2 * g + rel // 4
                blk = rel % 4
                if j < t:
                    moff[0, g * 8 + rel] = PAL_KEEP
                elif j == t:
                    moff[0, g * 8 + rel] = PAL_TRI0 - 128 * blk
                else:
                    moff[0, g * 8 + rel] = PAL_DROP
        in_maps.append(dict(xt=_tile_cols(xt), xq=_tile_cols(xt[:, cols]),
                            wkv=wkv_h, wq=wq_h, moff=moff, tri=tri, ident=ident))
    return in_maps


def assemble(results):
    out = np.empty((B, S, DV), dtype=np.float32)
    for core in range(8):
        b, p = core // 2, core % 2
        o = results[core]["out"]                      # [NG, 128, 4*64]
        for g in range(NG):
            t = TQ[p][g]
            # query q = blk*128 + p_row lives at o[g][p_row, blk*64:(blk+1)*64]
            blk_view = o[g].reshape(128, 4, DV).transpose(1, 0, 2)   # [blk,p,dv]
            out[b, t * QT:(t + 1) * QT, :] = blk_view.reshape(QT, DV)
    return out


_cache = {}


def _get_nc(kiter=1):
    if kiter not in _cache:
        _cache[kiter] = build(kiter)
    return _cache[kiter]


def run(x, Wq, Wk, Wv, kiter=1):
    nc = _get_nc(kiter)
    in_maps = make_inputs(x, Wq, Wk, Wv)
    res = run_bass_kernel_spmd(nc, in_maps, list(range(8)))
    return assemble(res.results)


def kernel(x, Wq, Wk, Wv):
    x = np.asarray(x, dtype=np.float32)
    return run(x, np.asarray(Wq, np.float32), np.asarray(Wk, np.float32),
               np.asarray(Wv, np.float32))
